# revision 1
# baseline (speedup 1.0000x reference)
"""Trainium2 Bass kernel for nn_ABCFramework_17755394802208.

Conv backbone (6x 3x3 SAME convs + 2 maxpools, 256^2 -> 64^2) feeding full
self-attention over N=4096 tokens with a Swin-style relative-position bias.

Sharding: 8 cores = (batch b in {0,1}) x (head h in {0..3}); each core runs the
conv backbone for its batch, projects q/k/v for its head, and computes full
attention for its (b, h). Output slices are gathered/reassembled on host.

The relative-position bias uses the block-Toeplitz structure of the Swin bias:
bias[n, m] = C[xn-xm+63, yn-ym+63] with C a 127x127 slice of the bias table.
Each SBUF partition p (key m within a 128-chunk) holds a contiguous 65KB
window W[p, t] = cfbuf[off_p + t] of the compact table, so every bias tile of
S^T is a plain strided slice of W; the add happens on the TensorEngine by
accumulating identity @ W_slice into the logits PSUM.
"""
import sys

sys.path.insert(0, '/opt/trn_rl_repo')

import numpy as np

NUM_HEADS = 4
DIM_HEAD = 64
TABLE_M = 160
B = 2
N = 4096          # tokens (64 x 64)
GRID = 64
NCH = 32          # m-chunks of 128 keys
NQC = 8           # n-chunks of 512 queries
CF = 8192         # 127 * 64 compact-table window length
CFBUF = 16576     # padded DRAM buffer (191 front pad + CF + tail pad)
SUPER = 2048      # im2col super-chunk (4 matmul chunks)

# conv layer configs: (Cin, Cout, H, W)
CONVS = [(1, 3, 256, 256), (3, 3, 256, 256),
         (3, 6, 128, 128), (6, 6, 128, 128),
         (6, 9, 64, 64), (9, 9, 64, 64)]


def _lay(Hdim, Wdim):
    Wp = Wdim + 2
    G = Wp + 1
    return Wp, G, Hdim * Wp, (Hdim + 2) * Wp + 2 * G  # Wp, guard, npix, buflen


_BUILD_CACHE = {}


def _build(trace_unused=False):
    if 'nc' in _BUILD_CACHE:
        return _BUILD_CACHE['nc']
    import concourse.bass as bass
    import concourse.mybir as mybir
    import concourse.tile as tile
    from concourse import bacc
    from concourse.masks import make_identity

    F32 = mybir.dt.float32
    F32R = mybir.dt.float32r
    AF = mybir.ActivationFunctionType

    import os
    DBG = os.environ.get("KDBG", "") == "1"
    nc = bacc.Bacc("TRN2", target_bir_lowering=False, debug=False, num_devices=8)

    # ---- external inputs (per-core shards prepared on host) ----
    _, _, _, BUF1 = _lay(256, 256)
    x_d = nc.dram_tensor("x", [1, BUF1], F32R, kind="ExternalInput")
    w_d, b_d = [], []
    for i, (ci, co, _, _) in enumerate(CONVS):
        w_d.append(nc.dram_tensor(f"w{i}", [ci * 9, co], F32R, kind="ExternalInput"))
        b_d.append(nc.dram_tensor(f"b{i}", [co], F32, kind="ExternalInput"))
    wq_d = nc.dram_tensor("wq", [9, 64], F32R, kind="ExternalInput")
    wk_d = nc.dram_tensor("wk", [9, 64], F32R, kind="ExternalInput")
    wv_d = nc.dram_tensor("wv", [9, 64], F32R, kind="ExternalInput")
    wa_d = nc.dram_tensor("watlas", [128, CF], F32R, kind="ExternalInput")
    out_d = nc.dram_tensor("out", [64, N], F32, kind="ExternalOutput")
    if DBG:
        dtok_d = nc.dram_tensor("dtok", [9, N], F32, kind="ExternalOutput")
        dq_d = nc.dram_tensor("dq", [64, N], F32, kind="ExternalOutput")
        dk_d = nc.dram_tensor("dk", [64, N], F32, kind="ExternalOutput")
        dv_d = nc.dram_tensor("dv", [128, NCH * 65], F32, kind="ExternalOutput")
        dm_d = []
        for i, (ci, co, Hd, Wd) in enumerate(CONVS):
            _, _, _, bl = _lay(Hd, Wd)
            dm_d.append(nc.dram_tensor(f"dm{i}", [co, bl], F32R, kind="ExternalOutput"))
        _, _, _, _bl3 = _lay(128, 128)
        dp2_d = nc.dram_tensor("dp2", [3, _bl3], F32R, kind="ExternalOutput")
        _, _, _, _bl5 = _lay(64, 64)
        dp4_d = nc.dram_tensor("dp4", [6, _bl5], F32R, kind="ExternalOutput")

    with tile.TileContext(nc) as tc:
        with tc.tile_pool(name="const", bufs=1) as const, \
             tc.tile_pool(name="work", bufs=(2 if DBG else 3)) as work, \
             tc.tile_pool(name="dram", bufs=1, space="DRAM") as dram:

            # ---------------- constants ----------------
            wt, bt = [], []
            for i, (ci, co, _, _) in enumerate(CONVS):
                w_t = const.tile([ci * 9, co], F32R, tag=f"w{i}")
                nc.sync.dma_start(out=w_t, in_=w_d[i][:, :])
                wt.append(w_t)
                b_t = const.tile([co, 1], F32, tag=f"b{i}")
                nc.sync.dma_start(out=b_t, in_=b_d[i][:, None])
                bt.append(b_t)
            wq_t = const.tile([9, 64], F32R, tag="wq")
            wk_t = const.tile([9, 64], F32R, tag="wk")
            wv_t = const.tile([9, 64], F32R, tag="wv")
            nc.sync.dma_start(out=wq_t, in_=wq_d[:, :])
            nc.sync.dma_start(out=wk_t, in_=wk_d[:, :])
            nc.sync.dma_start(out=wv_t, in_=wv_d[:, :])

            ident_f = const.tile([128, 128], F32, tag="idf")
            make_identity(nc, ident_f)
            ident = const.tile([128, 128], F32R, tag="id")
            nc.scalar.copy(out=ident, in_=ident_f)

            ones_f = const.tile([128, 1], F32, tag="onf")
            nc.vector.memset(ones_f, 1.0)
            ones_r = const.tile([1, 64], F32R, tag="onr")
            onesf64 = const.tile([1, 64], F32, tag="onf64")
            nc.vector.memset(onesf64, 1.0)
            nc.scalar.copy(out=ones_r, in_=onesf64)

            zeros = const.tile([9, 768], F32R, tag="zeros")
            nc.vector.memset(zeros.bitcast(F32), 0.0)

            # ---- bias atlas W (host-prepared sliding windows) ----
            W = const.tile([128, CF], F32R, tag="W")
            nc.sync.dma_start(out=W, in_=wa_d[:, :])

            # ---------------- conv backbone ----------------
            maps = []
            for i, (ci, co, Hd, Wd) in enumerate(CONVS):
                _, _, _, bl = _lay(Hd, Wd)
                maps.append(dram.tile([co, bl], F32R, tag=f"m{i}", name=f"m{i}"))
            _, _, _, bl3 = _lay(128, 128)
            p2 = dram.tile([3, bl3], F32R, tag="p2")
            _, _, _, bl5 = _lay(64, 64)
            p4 = dram.tile([6, bl5], F32R, tag="p4")

            def conv_layer(inten, Cin, Cout, Hd, Wd, w_t, b_t, outten, li):
                Wp, G, NPIX, bl_in = _lay(Hd, Wd)
                _, _, _, bl_out = _lay(Hd, Wd)
                nsup = (NPIX + SUPER - 1) // SUPER
                for si in range(nsup):
                    q0 = si * SUPER
                    sl = min(SUPER, NPIX - q0)
                    col = work.tile([Cin * 9, SUPER], F32R, tag="col")
                    for c in range(Cin):
                        base = c * bl_in + G + q0 - 1
                        src = bass.AP(tensor=inten, offset=base,
                                      ap=[[Wp, 3], [1, 3], [1, sl]])
                        nc.sync.dma_start(out=col[c * 9:(c + 1) * 9, 0:sl], in_=src)
                    for hb in range(0, sl, 2048):
                        hl = min(2048, sl - hb)
                        ro = work.tile([Cout, 2048], F32R, tag="ro", bufs=2)
                        nsub = (hl + 511) // 512
                        for k in range(nsub):
                            o0 = k * 512
                            ol = min(512, hl - o0)
                            pst = psc.tile([Cout, 512], F32, tag="cps")
                            nc.tensor.matmul(pst[:, 0:ol], w_t,
                                             col[:, hb + o0:hb + o0 + ol],
                                             start=True, stop=True)
                            h1 = ol // 2
                            if h1 > 0:
                                nc.scalar.activation(out=ro[:, o0:o0 + h1],
                                                     in_=pst[:, 0:h1], func=AF.Relu,
                                                     bias=b_t, scale=1.0)
                            nc.vector.scalar_tensor_tensor(
                                out=ro[:, o0 + h1:o0 + ol], in0=pst[:, h1:ol],
                                scalar=b_t, in1=zeros[0:Cout, 0:ol - h1],
                                op0=mybir.AluOpType.add, op1=mybir.AluOpType.max)
                        dst = bass.AP(tensor=outten, offset=G + Wp + q0 + hb,
                                      ap=[[bl_out, Cout], [1, hl]])
                        nc.gpsimd.dma_start(out=dst, in_=ro[0:Cout, 0:hl])
                # guard zeroing of outten
                gl = G + Wp
                dst = bass.AP(tensor=outten, offset=0, ap=[[bl_out, Cout], [1, gl]])
                nc.gpsimd.dma_start(out=dst, in_=zeros[0:Cout, 0:gl])
                dst = bass.AP(tensor=outten, offset=G + Wp * (Hd + 1),
                              ap=[[bl_out, Cout], [1, gl]])
                nc.gpsimd.dma_start(out=dst, in_=zeros[0:Cout, 0:gl])
                for gc in (0, Wp - 1):
                    dst = bass.AP(tensor=outten, offset=G + Wp + gc,
                                  ap=[[bl_out, Cout], [Wp, Hd]])
                    nc.gpsimd.dma_start(out=dst, in_=zeros[0:Cout, 0:Hd])

            def pool_layer(inten, C, Hd, Wd, outten):
                Wp, G, _, bl_in = _lay(Hd, Wd)
                H2, W2 = Hd // 2, Wd // 2
                Wp2, G2, _, bl_out = _lay(H2, W2)
                P2 = 128 // C  # row-pairs per chunk (per channel)
                for r0 in range(0, H2, P2):
                    rp = min(P2, H2 - r0)
                    t3 = work.tile([C * P2, 2, Wd], F32R, tag="plin")
                    for c in range(C):
                        src = bass.AP(tensor=inten,
                                      offset=c * bl_in + G + (2 * r0 + 1) * Wp + 1,
                                      ap=[[2 * Wp, rp], [Wp, 2], [1, Wd]])
                        nc.sync.dma_start(out=t3[c * P2:c * P2 + rp, :, :], in_=src)
                    m1 = work.tile([C * P2, 2, W2], F32R, tag="plw")
                    nc.vector.tensor_max(m1[:, :, :], t3[:, :, 0::2], t3[:, :, 1::2])
                    m2 = work.tile([C * P2, W2], F32R, tag="plh")
                    nc.vector.tensor_max(m2[:, :], m1[:, 0, :], m1[:, 1, :])
                    for c in range(C):
                        dst = bass.AP(tensor=outten,
                                      offset=c * bl_out + G2 + (r0 + 1) * Wp2 + 1,
                                      ap=[[Wp2, rp], [1, W2]])
                        nc.gpsimd.dma_start(out=dst, in_=m2[c * P2:c * P2 + rp, :])
                # guard zeroing of outten
                gl = G2 + Wp2
                dst = bass.AP(tensor=outten, offset=0, ap=[[bl_out, C], [1, gl]])
                nc.gpsimd.dma_start(out=dst, in_=zeros[0:C, 0:gl])
                dst = bass.AP(tensor=outten, offset=G2 + Wp2 * (H2 + 1),
                              ap=[[bl_out, C], [1, gl]])
                nc.gpsimd.dma_start(out=dst, in_=zeros[0:C, 0:gl])
                for gc in (0, Wp2 - 1):
                    dst = bass.AP(tensor=outten, offset=G2 + Wp2 + gc,
                                  ap=[[bl_out, C], [Wp2, H2]])
                    nc.gpsimd.dma_start(out=dst, in_=zeros[0:C, 0:H2])

            scope_conv = nc.named_scope("conv"); scope_conv.__enter__()
            with tc.tile_pool(name="psc", bufs=4, space="PSUM") as psc:
                conv_layer(x_d, 1, 3, 256, 256, wt[0], bt[0], maps[0].tensor, 0)
                conv_layer(maps[0].tensor, 3, 3, 256, 256, wt[1], bt[1], maps[1].tensor, 1)
                pool_layer(maps[1].tensor, 3, 256, 256, p2.tensor)
                conv_layer(p2.tensor, 3, 6, 128, 128, wt[2], bt[2], maps[2].tensor, 2)
                conv_layer(maps[2].tensor, 6, 6, 128, 128, wt[3], bt[3], maps[3].tensor, 3)
                pool_layer(maps[3].tensor, 6, 128, 128, p4.tensor)
                conv_layer(p4.tensor, 6, 9, 64, 64, wt[4], bt[4], maps[4].tensor, 4)
                conv_layer(maps[4].tensor, 9, 9, 64, 64, wt[5], bt[5], maps[5].tensor, 5)

            scope_conv.__exit__(None, None, None)
            scope_qkv = nc.named_scope("qkv"); scope_qkv.__enter__()
            # ---------------- tokens + q/k/v ----------------
            tokT = const.tile([9, N], F32R, tag="tok")
            Wp5, G5, _, bl5_ = _lay(64, 64)
            src = bass.AP(tensor=maps[5].tensor, offset=G5 + Wp5 + 1,
                          ap=[[bl5_, 9], [Wp5, 64], [1, 64]])
            nc.sync.dma_start(out=tokT.rearrange("c (h w) -> c h w", w=64), in_=src)

            qT = const.tile([64, N], F32R, tag="qT")
            kT = const.tile([64, N], F32R, tag="kT")
            v_sb = const.tile([128, NCH, 65], F32R, tag="v")

            with tc.tile_pool(name="psq", bufs=2, space="PSUM") as psq:
                for j in range(NQC):
                    ps_q = psq.tile([64, 512], F32, tag="qps")
                    nc.tensor.matmul(ps_q, wq_t, tokT[:, j * 512:(j + 1) * 512],
                                     start=True, stop=True)
                    nc.scalar.activation(out=qT[:, j * 512:(j + 1) * 512], in_=ps_q,
                                         func=AF.Copy, scale=float(DIM_HEAD) ** -0.5)
                    ps_k = psq.tile([64, 512], F32, tag="kps")
                    nc.tensor.matmul(ps_k, wk_t, tokT[:, j * 512:(j + 1) * 512],
                                     start=True, stop=True)
                    nc.scalar.copy(out=kT[:, j * 512:(j + 1) * 512], in_=ps_k)
                for c in range(NCH):
                    ps_v = psq.tile([128, 64], F32, tag="vps")
                    nc.tensor.matmul(ps_v, tokT[:, c * 128:(c + 1) * 128], wv_t,
                                     start=True, stop=True)
                    nc.scalar.copy(out=v_sb[:, c, 0:64], in_=ps_v)
                    nc.vector.tensor_copy(v_sb[:, c, 64:65], ones_f)

            if DBG:
                for i in range(6):
                    nc.sync.dma_start(out=dm_d[i][:, :], in_=maps[i][:, :])
                nc.sync.dma_start(out=dp2_d[:, :], in_=p2[:, :])
                nc.sync.dma_start(out=dp4_d[:, :], in_=p4[:, :])
                dt_ = const.tile([9, N], F32, tag="dbg")
                nc.vector.tensor_copy(dt_, tokT)
                nc.sync.dma_start(out=dtok_d[:, :], in_=dt_)
                dq_ = const.tile([64, N], F32, tag="dbg")
                nc.vector.tensor_copy(dq_, qT)
                nc.sync.dma_start(out=dq_d[:, :], in_=dq_)
                dk_ = const.tile([64, N], F32, tag="dbg")
                nc.vector.tensor_copy(dk_, kT)
                nc.sync.dma_start(out=dk_d[:, :], in_=dk_)
                dv_ = const.tile([128, NCH * 65], F32, tag="dbg")
                nc.vector.tensor_copy(dv_, v_sb.rearrange("p c d -> p (c d)"))
                nc.sync.dma_start(out=dv_d[:, :], in_=dv_)

            scope_qkv.__exit__(None, None, None)
            # ---------------- attention ----------------
            Wv = W
            scope_attn = nc.named_scope("attn"); scope_attn.__enter__()
            with tc.tile_pool(name="pss", bufs=4, space="PSUM") as pss, \
                 tc.tile_pool(name="psa", bufs=2, space="PSUM") as psa, \
                 tc.tile_pool(name="psm", bufs=2, space="PSUM") as psm:
                for j in range(NQC):
                    acc = psa.tile([65, 512], F32, tag="acc")
                    for c in range(NCH):
                        s_ps = pss.tile([128, 512], F32, tag="s")
                        nc.tensor.matmul(s_ps, kT[:, c * 128:(c + 1) * 128],
                                         qT[:, j * 512:(j + 1) * 512],
                                         start=True, stop=True)
                        s0 = (8 * j - 2 * c + 63) * 64
                        lg = work.tile([128, 512], F32, tag="lg", bufs=2)
                        nc.vector.tensor_add(lg, s_ps, Wv[:, s0:s0 + 512])
                        at = work.tile([128, 512], F32R, tag="at", bufs=4)
                        nc.scalar.activation(out=at, in_=lg, func=AF.Exp)
                        nc.tensor.matmul(acc, v_sb[:, c, :], at,
                                         start=(c == 0), stop=(c == NCH - 1))
                    # epilogue: divide by the attention sums (row 64 of acc)
                    sums = work.tile([1, 512], F32, tag="sums", bufs=2)
                    nc.scalar.copy(out=sums, in_=acc[64:65, :])
                    rcp_f = work.tile([1, 512], F32, tag="rcpf", bufs=2)
                    nc.vector.reciprocal_approx_fast(out=rcp_f, in_=sums)
                    rcp = work.tile([1, 512], F32R, tag="rcp", bufs=2)
                    nc.scalar.copy(out=rcp, in_=rcp_f)
                    bc_ps = psm.tile([64, 512], F32, tag="bc")
                    nc.tensor.matmul(bc_ps, ones_r, rcp, start=True, stop=True)
                    bc_sb = work.tile([64, 512], F32, tag="bcs", bufs=2)
                    nc.scalar.copy(out=bc_sb, in_=bc_ps)
                    res = work.tile([64, 512], F32, tag="res", bufs=2)
                    nc.vector.tensor_mul(res, acc[0:64, :], bc_sb)
                    nc.sync.dma_start(out=out_d[:, j * 512:(j + 1) * 512], in_=res)
            scope_attn.__exit__(None, None, None)

    nc.finalize()
    _BUILD_CACHE['nc'] = nc
    return nc


def _prep_inputs(inputs):
    """Build the 8 per-core input maps (pure slicing/layout, no math)."""
    x = np.asarray(inputs['x'], dtype=np.float32)
    qkv_w = np.asarray(inputs['qkv_w'], dtype=np.float32)
    table = np.asarray(inputs['bias_table'], dtype=np.float32)

    _, _, _, BUF1 = _lay(256, 256)
    xbufs = []
    for b in range(B):
        pad = np.zeros((258, 258), np.float32)
        pad[1:257, 1:257] = x[b, 0]
        buf = np.zeros((1, BUF1), np.float32)
        g1 = 258 + 1 + 2 * 0  # G = Wp+1 = 259
        G = 259
        buf[0, G:G + 258 * 258] = pad.reshape(-1)
        xbufs.append(buf)

    wts, bts = [], []
    for i in range(6):
        w = np.asarray(inputs[f'conv{i + 1}_w'], dtype=np.float32)
        bias = np.asarray(inputs[f'conv{i + 1}_b'], dtype=np.float32)
        wts.append(np.ascontiguousarray(
            w.transpose(1, 2, 3, 0).reshape(-1, w.shape[0])))
        bts.append(np.ascontiguousarray(bias))

    atlases = []
    for h in range(NUM_HEADS):
        tab = table[:, h].reshape(2 * TABLE_M - 1, 2 * TABLE_M - 1)
        C = tab[96:96 + 127, 96:96 + 127]  # [127, 127]
        tmp = np.zeros((127, 128), np.float32)
        tmp[:, :127] = C
        cfbuf = np.zeros(191 + 16256 + 129, np.float32)
        cfbuf[191:191 + 16256] = tmp.reshape(-1)
        sw = np.lib.stride_tricks.sliding_window_view(cfbuf, 16256)
        p = np.arange(128)
        offs = 254 - (p % 64) - 128 * (p // 64)
        full = sw[offs]                                   # [128, 127*128]
        a2 = full.reshape(128, 127, 128)[:, :, 0:64].reshape(128, 127 * 64)
        atl = np.zeros((128, CF), np.float32)
        atl[:, :127 * 64] = a2
        atlases.append(atl)

    in_maps = []
    for core in range(8):
        b, h = core // 4, core % 4
        m = {"x": xbufs[b], "watlas": atlases[h]}
        for i in range(6):
            m[f"w{i}"] = wts[i]
            m[f"b{i}"] = bts[i]
        m["wq"] = np.ascontiguousarray(qkv_w[h * 64:(h + 1) * 64, :].T)
        m["wk"] = np.ascontiguousarray(qkv_w[256 + h * 64:256 + (h + 1) * 64, :].T)
        m["wv"] = np.ascontiguousarray(qkv_w[512 + h * 64:512 + (h + 1) * 64, :].T)
        in_maps.append(m)
    return in_maps


def kernel(_trace=False, **inputs):
    from concourse.bass_utils import run_bass_kernel_spmd
    nc = _build()
    in_maps = _prep_inputs(inputs)
    import os
    tdir = os.environ.get("KTRACE_DIR")
    if tdir:
        os.makedirs(tdir, exist_ok=True)
    res = run_bass_kernel_spmd(nc, in_maps, core_ids=list(range(8)),
                               trace=_trace, tmpdir=tdir)
    if _trace:
        kernel.last_exec_ns = res.exec_time_ns
        kernel.last_results = res
    # assemble: core -> (b, h): [64(d), 4096(n)]
    O = np.stack([np.stack([res.results[b * 4 + h]["out"] for h in range(4)])
                  for b in range(B)])                      # [B, H, 64, N]
    out = O.transpose(0, 3, 1, 2).reshape(B, N, NUM_HEADS * DIM_HEAD)
    out = out.reshape(B, GRID, GRID, NUM_HEADS * DIM_HEAD)
    shift = int(np.asarray(inputs['window_size'])) // 2
    out = np.roll(out, shift=(-shift, -shift), axis=(1, 2))
    return out.astype(np.float32)



# revision 9
# speedup vs baseline: 1.5322x; 1.5322x over previous
"""Trainium2 Bass kernel for nn_ABCFramework_17755394802208.

Conv backbone (6x 3x3 SAME convs + 2 maxpools, 256^2 -> 64^2) feeding full
self-attention over N=4096 tokens with a Swin-style relative-position bias.

Sharding: 8 cores = (batch b in {0,1}) x (head h in {0..3}); each core runs the
conv backbone for its batch, projects q/k/v for its head, and computes full
attention for its (b, h). Output slices are gathered/reassembled on host.

Conv formulation: each matmul processes G image rows at once. The moving tile
holds rows (ky, ci, g) x (strip s, padded col xp); the stationary weights are
block-diagonal [3*Cin*G, Cout*G] (one block per g), one stationary per kx tap,
accumulated over kx in PSUM. All matmuls run in bf16.

Attention: logits S^T = K_c^T Q_j in PSUM (bf16 matmul), exp on the Scalar
engine, then the relative-position bias is applied multiplicatively on the
Vector engine: exp(s+b) = exp(s) * EB where EB = exp(bias atlas) is computed
once on-chip. AV accumulates in PSUM with an extra ones-row in V giving the
softmax denominators.
"""
import sys

sys.path.insert(0, '/opt/trn_rl_repo')

import numpy as np

try:
    from ml_dtypes import bfloat16 as BF16_NP
except ImportError:  # pragma: no cover
    import jax.numpy as _jnp
    BF16_NP = _jnp.bfloat16

NUM_HEADS = 4
DIM_HEAD = 64
TABLE_M = 160
B = 2
N = 4096          # tokens (64 x 64)
GRID = 64
NCH = 32          # m-chunks of 128 keys
NQC = 8           # n-chunks of 512 queries
CF = 8192         # 127 * 64 compact-table window length

# conv layer configs: (Cin, Cout, H, W, G rows-per-group)
CONVS = [(1, 3, 256, 256, 42), (3, 3, 256, 256, 14),
         (3, 6, 128, 128, 14), (6, 6, 128, 128, 7),
         (6, 9, 64, 64, 7), (9, 9, 64, 64, 4)]

# plane name -> (C, H, W, slack rows below the bottom guard)
PLANES = {
    'x':  (1, 256, 256, 38),
    'm1': (3, 256, 256, 37),
    'm2': (3, 256, 256, 9),
    'p1': (3, 128, 128, 12),
    'm3': (6, 128, 128, 11),
    'm4': (6, 128, 128, 4),
    'p2': (6, 64, 64, 6),
    'm5': (9, 64, 64, 5),
}
# conv wiring: (layer idx, in plane, out plane or 'tok')
WIRE = [(0, 'x', 'm1'), (1, 'm1', 'm2'), (2, 'p1', 'm3'),
        (3, 'm3', 'm4'), (4, 'p2', 'm5'), (5, 'm5', 'tok')]
POOLS = [('m2', 'p1'), ('m4', 'p2')]
BOTTOM_FIX = {'m1', 'm3', 'm5'}        # planes needing post-layer guard re-zero


def _lay(H, W, slack):
    Wp = W + 2
    Goff = Wp + 1
    rows = H + 2 + slack
    return Wp, Goff, rows * Wp + 2 * Goff  # Wp, guard offset, buflen


def _chunks(Sf, R, W):
    ns_max = 512 // W
    out = []
    s = 0
    while s < Sf:
        ns = min(ns_max, Sf - s)
        out.append((s, ns))
        s += ns
    if R:
        out.append((Sf, 1))
    return out


_BUILD_CACHE = {}


def _build():
    if 'nc' in _BUILD_CACHE:
        return _BUILD_CACHE['nc']
    import concourse.bass as bass
    import concourse.mybir as mybir
    import concourse.tile as tile
    from concourse import bacc

    F32 = mybir.dt.float32
    BF16 = mybir.dt.bfloat16
    AF = mybir.ActivationFunctionType
    ALU = mybir.AluOpType

    nc = bacc.Bacc("TRN2", target_bir_lowering=False, debug=False, num_devices=8)

    # ---- external inputs (per-core shards prepared on host) ----
    _, _, BLX = _lay(*PLANES['x'][1:])
    x_d = nc.dram_tensor("x", [1, BLX], BF16, kind="ExternalInput")
    wk_d, bx_d = [], []
    for i, (ci, co, _, _, G) in enumerate(CONVS):
        rows, cols = 3 * ci * G, co * G
        wk_d.append([nc.dram_tensor(f"w{i}_{kx}", [rows, cols], BF16,
                                    kind="ExternalInput") for kx in range(3)])
        bx_d.append(nc.dram_tensor(f"bx{i}", [cols], F32, kind="ExternalInput"))
    wq_d = nc.dram_tensor("wq", [9, 64], BF16, kind="ExternalInput")
    wkk_d = nc.dram_tensor("wk", [9, 64], BF16, kind="ExternalInput")
    wv_d = nc.dram_tensor("wv", [9, 64], BF16, kind="ExternalInput")
    wa_d = nc.dram_tensor("watlas", [128, CF], BF16, kind="ExternalInput")
    out_d = nc.dram_tensor("out", [64, N], F32, kind="ExternalOutput")

    with tile.TileContext(nc) as tc:
        with tc.tile_pool(name="const", bufs=1) as const, \
             tc.tile_pool(name="work", bufs=2) as work, \
             tc.tile_pool(name="dram", bufs=1, space="DRAM") as dram:

            # ---------------- constants ----------------
            wkt, bxt = [], []
            for i, (ci, co, _, _, G) in enumerate(CONVS):
                rows, cols = 3 * ci * G, co * G
                trip = []
                for kx in range(3):
                    t = const.tile([rows, cols], BF16, tag=f"w{i}_{kx}",
                                   name=f"w{i}_{kx}")
                    nc.sync.dma_start(out=t, in_=wk_d[i][kx][:, :])
                    trip.append(t)
                wkt.append(trip)
                bt = const.tile([cols, 1], F32, tag=f"bx{i}", name=f"bx{i}")
                nc.sync.dma_start(out=bt, in_=bx_d[i][:, None])
                bxt.append(bt)
            wq_t = const.tile([9, 64], BF16, tag="wq")
            wkk_t = const.tile([9, 64], BF16, tag="wkk")
            wv_t = const.tile([9, 64], BF16, tag="wv")
            nc.sync.dma_start(out=wq_t, in_=wq_d[:, :])
            nc.sync.dma_start(out=wkk_t, in_=wkk_d[:, :])
            nc.sync.dma_start(out=wv_t, in_=wv_d[:, :])

            zeros_bf = const.tile([128, 512], BF16, tag="zbf")
            nc.vector.memset(zeros_bf, 0.0)
            ones_bf = const.tile([1, 64], BF16, tag="obf")
            nc.vector.memset(ones_bf, 1.0)

            # bias atlas -> EB = exp(atlas), computed once on-chip
            Wt = const.tile([128, CF], BF16, tag="W")
            nc.sync.dma_start(out=Wt, in_=wa_d[:, :])
            EB = const.tile([128, CF], BF16, tag="EB")
            nc.scalar.activation(out=EB, in_=Wt, func=AF.Exp)

            # ---------------- DRAM planes + zero scratch ----------------
            geom, plane = {}, {}
            for nm, (C, H, Wd, slack) in PLANES.items():
                geom[nm] = _lay(H, Wd, slack)
                if nm != 'x':
                    plane[nm] = dram.tile([C, geom[nm][2]], BF16, tag=nm, name=nm)
            tok_d = dram.tile([9, N], BF16, tag="tok", name="tok")
            zs = dram.tile([1, 32768], BF16, tag="zs", name="zs")
            nc.sync.dma_start(
                out=bass.AP(tensor=zs.tensor, offset=0, ap=[[256, 128], [1, 256]]),
                in_=zeros_bf[0:128, 0:256])

            def zfill(tensor, offset, ap):
                total = 1
                for _, cnt in ap:
                    total *= cnt
                assert total <= 32768, total
                nc.sync.dma_start(
                    out=bass.AP(tensor=tensor, offset=offset, ap=ap),
                    in_=bass.AP(tensor=zs.tensor, offset=0, ap=[[1, total]]))

            # initial guards: top strip + both columns for every on-chip plane,
            # bottom strip for pool outputs (pools never overwrite them)
            for nm in ['m1', 'm2', 'p1', 'm3', 'm4', 'p2', 'm5']:
                C, H, Wd, slack = PLANES[nm]
                Wp, Goff, bl = geom[nm]
                t = plane[nm].tensor
                zfill(t, 0, [[bl, C], [1, Goff + Wp]])
                zfill(t, Goff + Wp, [[bl, C], [Wp, H + 1 + slack]])
                zfill(t, Goff + 2 * Wp - 1, [[bl, C], [Wp, H + 1 + slack]])
                if nm in ('p1', 'p2'):
                    zfill(t, Goff + (H + 1) * Wp,
                          [[bl, C], [1, (slack + 1) * Wp + Goff]])

            # ---------------- conv backbone ----------------
            def conv_layer(li, in_nm, out_nm):
                Cin, Cout, H, Wd, G = CONVS[li]
                Wp, Goff, bl = geom[in_nm]
                Sf, R = H // G, H % G
                S_tot = Sf + (1 if R else 0)
                rows, cols = 3 * Cin * G, Cout * G
                in_t = x_d if in_nm == 'x' else plane[in_nm].tensor

                mov = work.tile([rows, S_tot, Wp], BF16, tag="mov", bufs=2,
                                name=f"mov{li}")
                lengs = [nc.sync, nc.scalar]
                for ky in range(3):
                    for ci in range(Cin):
                        src = bass.AP(tensor=in_t,
                                      offset=ci * bl + Goff + ky * Wp,
                                      ap=[[Wp, G], [G * Wp, S_tot], [1, Wp]])
                        pb = (ky * Cin + ci) * G
                        lengs[(ky * Cin + ci) % 2].dma_start(
                            out=mov[pb:pb + G, :, :], in_=src)

                out_t = work.tile([cols, S_tot, Wd], BF16, tag="out", bufs=2,
                                  name=f"out{li}")
                chs = _chunks(Sf, R, Wd)
                for g0 in range(0, len(chs), 4):
                    grp = chs[g0:g0 + 4]
                    pts = [psc.tile([cols, 512], F32, tag="cps",
                                    name=f"cps{li}_{g0}_{gi}")
                           for gi in range(len(grp))]
                    for kx in range(3):
                        for pt, (s0, ns) in zip(pts, grp):
                            nc.tensor.matmul(pt[:, 0:ns * Wd], wkt[li][kx],
                                             mov[:, s0:s0 + ns, kx:kx + Wd],
                                             start=(kx == 0), stop=(kx == 2))
                    for pt, (s0, ns) in zip(pts, grp):
                        nc.vector.scalar_tensor_tensor(
                            out=out_t[:, s0:s0 + ns, 0:Wd], in0=pt[:, 0:ns * Wd],
                            scalar=bxt[li], in1=zeros_bf[0:cols, 0:ns * Wd],
                            op0=ALU.add, op1=ALU.max)

                wengs = [nc.gpsimd, nc.sync, nc.scalar]
                if out_nm == 'tok':
                    for co in range(Cout):
                        dst = bass.AP(tensor=tok_d.tensor, offset=co * N,
                                      ap=[[64, G], [G * 64, Sf], [1, 64]])
                        wengs[co % 3].dma_start(
                            out=dst, in_=out_t[co * G:(co + 1) * G, 0:Sf, :])
                    return
                Wpo, Goffo, blo = geom[out_nm]
                slo = PLANES[out_nm][3]
                ot = plane[out_nm].tensor
                # one write per output channel; with a partial strip (R>0) the
                # garbage rows g>=R of strip Sf spill into the slack region,
                # re-zeroed below for planes whose consumer reads the guard row
                for co in range(Cout):
                    dst = bass.AP(tensor=ot, offset=co * blo + Goffo + Wpo + 1,
                                  ap=[[Wpo, G], [G * Wpo, S_tot], [1, Wd]])
                    wengs[co % 3].dma_start(
                        out=dst, in_=out_t[co * G:(co + 1) * G, 0:S_tot, :])
                if out_nm in BOTTOM_FIX:
                    C_, H_, _, _ = PLANES[out_nm]
                    total = (slo + 1) * Wpo + Goffo
                    nc.gpsimd.dma_start(
                        out=bass.AP(tensor=ot, offset=Goffo + (H_ + 1) * Wpo,
                                    ap=[[blo, C_], [1, total]]),
                        in_=bass.AP(tensor=zs.tensor, offset=0,
                                    ap=[[1, C_ * total]]))

            def pool_layer(in_nm, out_nm):
                C, H, Wd, _ = PLANES[in_nm]
                Wp, Goff, bl = geom[in_nm]
                H2, W2 = H // 2, Wd // 2
                Wp2, Goff2, bl2 = geom[out_nm]
                it, ot = plane[in_nm].tensor, plane[out_nm].tensor
                P2 = 128 // C
                for r0 in range(0, H2, P2):
                    rp = min(P2, H2 - r0)
                    t3 = work.tile([128, 2, Wd], BF16, tag="pool", bufs=2)
                    for c in range(C):
                        src = bass.AP(tensor=it,
                                      offset=c * bl + Goff + (2 * r0 + 1) * Wp + 1,
                                      ap=[[2 * Wp, rp], [Wp, 2], [1, Wd]])
                        (nc.sync if c % 2 else nc.scalar).dma_start(
                            out=t3[c * P2:c * P2 + rp, :, :], in_=src)
                    m1t = work.tile([128, 2, W2], BF16, tag="plw", bufs=2)
                    nc.vector.tensor_max(m1t, t3[:, :, 0::2], t3[:, :, 1::2])
                    m2t = work.tile([128, W2], BF16, tag="plh", bufs=2)
                    nc.vector.tensor_max(m2t, m1t[:, 0, :], m1t[:, 1, :])
                    for c in range(C):
                        dst = bass.AP(tensor=ot,
                                      offset=c * bl2 + Goff2 + (r0 + 1) * Wp2 + 1,
                                      ap=[[Wp2, rp], [1, W2]])
                        (nc.gpsimd if c % 2 else nc.sync).dma_start(
                            out=dst, in_=m2t[c * P2:c * P2 + rp, :])

            scope_conv = nc.named_scope("conv"); scope_conv.__enter__()
            with tc.tile_pool(name="psc", bufs=8, space="PSUM") as psc:
                conv_layer(0, 'x', 'm1')
                conv_layer(1, 'm1', 'm2')
                pool_layer('m2', 'p1')
                conv_layer(2, 'p1', 'm3')
                conv_layer(3, 'm3', 'm4')
                pool_layer('m4', 'p2')
                conv_layer(4, 'p2', 'm5')
                conv_layer(5, 'm5', 'tok')
            scope_conv.__exit__(None, None, None)

            # ---------------- tokens + q/k/v ----------------
            scope_qkv = nc.named_scope("qkv"); scope_qkv.__enter__()
            tokT = const.tile([9, N], BF16, tag="tok")
            nc.sync.dma_start(out=tokT, in_=tok_d[:, :])

            qT = const.tile([64, N], BF16, tag="qT")
            kT = const.tile([64, N], BF16, tag="kT")
            v_sb = const.tile([128, NCH, 65], BF16, tag="v")
            nc.vector.memset(v_sb, 1.0)

            with tc.tile_pool(name="psq", bufs=2, space="PSUM") as psq:
                for j in range(NQC):
                    ps_q = psq.tile([64, 512], F32, tag="qps")
                    nc.tensor.matmul(ps_q, wq_t, tokT[:, j * 512:(j + 1) * 512],
                                     start=True, stop=True)
                    nc.scalar.activation(out=qT[:, j * 512:(j + 1) * 512],
                                         in_=ps_q, func=AF.Copy,
                                         scale=float(DIM_HEAD) ** -0.5)
                    ps_k = psq.tile([64, 512], F32, tag="kps")
                    nc.tensor.matmul(ps_k, wkk_t, tokT[:, j * 512:(j + 1) * 512],
                                     start=True, stop=True)
                    nc.scalar.activation(out=kT[:, j * 512:(j + 1) * 512],
                                         in_=ps_k, func=AF.Copy)
                for c in range(NCH):
                    ps_v = psq.tile([128, 64], F32, tag="vps")
                    nc.tensor.matmul(ps_v, tokT[:, c * 128:(c + 1) * 128], wv_t,
                                     start=True, stop=True)
                    nc.vector.tensor_copy(v_sb[:, c, 0:64], ps_v)
            scope_qkv.__exit__(None, None, None)

            # ---------------- attention ----------------
            scope_attn = nc.named_scope("attn"); scope_attn.__enter__()
            with tc.tile_pool(name="pss", bufs=4, space="PSUM") as pss, \
                 tc.tile_pool(name="psa", bufs=2, space="PSUM") as psa, \
                 tc.tile_pool(name="psm", bufs=2, space="PSUM") as psm:
                for j in range(NQC):
                    acc = psa.tile([65, 512], F32, tag="acc")
                    for c in range(NCH):
                        s_ps = pss.tile([128, 512], F32, tag="s")
                        nc.tensor.matmul(s_ps, kT[:, c * 128:(c + 1) * 128],
                                         qT[:, j * 512:(j + 1) * 512],
                                         start=True, stop=True)
                        at = work.tile([128, 512], BF16, tag="at", bufs=3)
                        nc.scalar.activation(out=at, in_=s_ps, func=AF.Exp)
                        atb = work.tile([128, 512], BF16, tag="atb", bufs=3)
                        s0 = (8 * j - 2 * c + 63) * 64
                        nc.vector.tensor_mul(atb, at, EB[:, s0:s0 + 512])
                        nc.tensor.matmul(acc, v_sb[:, c, :], atb,
                                         start=(c == 0), stop=(c == NCH - 1))
                    # epilogue: divide by the attention sums (row 64 of acc)
                    sums = work.tile([1, 512], F32, tag="sums", bufs=2)
                    nc.vector.tensor_copy(sums, acc[64:65, :])
                    rcp_f = work.tile([1, 512], F32, tag="rcpf", bufs=2)
                    nc.vector.reciprocal_approx_fast(out=rcp_f, in_=sums)
                    rcp = work.tile([1, 512], BF16, tag="rcp", bufs=2)
                    nc.vector.tensor_copy(rcp, rcp_f)
                    bc_ps = psm.tile([64, 512], F32, tag="bc")
                    nc.tensor.matmul(bc_ps, ones_bf, rcp, start=True, stop=True)
                    bc_sb = work.tile([64, 512], F32, tag="bcs", bufs=2)
                    nc.vector.tensor_copy(bc_sb, bc_ps)
                    res = work.tile([64, 512], F32, tag="res", bufs=2)
                    nc.vector.tensor_mul(res, acc[0:64, :], bc_sb)
                    nc.sync.dma_start(out=out_d[:, j * 512:(j + 1) * 512],
                                      in_=res)
            scope_attn.__exit__(None, None, None)

    nc.finalize()
    _BUILD_CACHE['nc'] = nc
    return nc


def _prep_inputs(inputs):
    """Build the 8 per-core input maps (layout/packing only)."""
    x = np.asarray(inputs['x'], dtype=np.float32)
    qkv_w = np.asarray(inputs['qkv_w'], dtype=np.float32)
    table = np.asarray(inputs['bias_table'], dtype=np.float32)

    Wp, Goff, BLX = _lay(*PLANES['x'][1:])
    rows_x = PLANES['x'][1] + 2 + PLANES['x'][3]
    xbufs = []
    for b in range(B):
        pad = np.zeros((rows_x, Wp), np.float32)
        pad[1:257, 1:257] = x[b, 0]
        buf = np.zeros((1, BLX), np.float32)
        buf[0, Goff:Goff + rows_x * Wp] = pad.reshape(-1)
        xbufs.append(buf.astype(BF16_NP))

    wks, bxs = [], []
    for i, (Cin, Cout, _, _, G) in enumerate(CONVS):
        w = np.asarray(inputs[f'conv{i + 1}_w'], dtype=np.float32)
        bias = np.asarray(inputs[f'conv{i + 1}_b'], dtype=np.float32)
        trip = []
        ar = np.arange(G)
        for kx in range(3):
            Wk = np.zeros((3 * Cin * G, Cout * G), np.float32)
            for ky in range(3):
                for ci in range(Cin):
                    r0 = (ky * Cin + ci) * G
                    for co in range(Cout):
                        Wk[r0 + ar, co * G + ar] = w[co, ci, ky, kx]
            trip.append(Wk.astype(BF16_NP))
        wks.append(trip)
        bxs.append(np.repeat(bias, G).astype(np.float32))

    atlases = []
    for h in range(NUM_HEADS):
        tab = table[:, h].reshape(2 * TABLE_M - 1, 2 * TABLE_M - 1)
        Ct = tab[96:96 + 127, 96:96 + 127]  # [127, 127]
        tmp = np.zeros((127, 128), np.float32)
        tmp[:, :127] = Ct
        cfbuf = np.zeros(191 + 16256 + 129, np.float32)
        cfbuf[191:191 + 16256] = tmp.reshape(-1)
        sw = np.lib.stride_tricks.sliding_window_view(cfbuf, 16256)
        p = np.arange(128)
        offs = 254 - (p % 64) - 128 * (p // 64)
        full = sw[offs]                                   # [128, 127*128]
        a2 = full.reshape(128, 127, 128)[:, :, 0:64].reshape(128, 127 * 64)
        atl = np.zeros((128, CF), np.float32)
        atl[:, :127 * 64] = a2
        atlases.append(atl.astype(BF16_NP))

    in_maps = []
    for core in range(8):
        b, h = core // 4, core % 4
        m = {"x": xbufs[b], "watlas": atlases[h]}
        for i in range(6):
            for kx in range(3):
                m[f"w{i}_{kx}"] = wks[i][kx]
            m[f"bx{i}"] = bxs[i]
        m["wq"] = np.ascontiguousarray(
            qkv_w[h * 64:(h + 1) * 64, :].T).astype(BF16_NP)
        m["wk"] = np.ascontiguousarray(
            qkv_w[256 + h * 64:256 + (h + 1) * 64, :].T).astype(BF16_NP)
        m["wv"] = np.ascontiguousarray(
            qkv_w[512 + h * 64:512 + (h + 1) * 64, :].T).astype(BF16_NP)
        in_maps.append(m)
    return in_maps


def kernel(_trace=False, **inputs):
    from concourse.bass_utils import run_bass_kernel_spmd
    nc = _build()
    in_maps = _prep_inputs(inputs)
    import os
    tdir = os.environ.get("KTRACE_DIR")
    if tdir:
        os.makedirs(tdir, exist_ok=True)
    res = run_bass_kernel_spmd(nc, in_maps, core_ids=list(range(8)),
                               trace=_trace, tmpdir=tdir)
    if _trace:
        kernel.last_exec_ns = res.exec_time_ns
        kernel.last_results = res
    # assemble: core -> (b, h): [64(d), 4096(n)]
    O = np.stack([np.stack([res.results[b * 4 + h]["out"] for h in range(4)])
                  for b in range(B)])                      # [B, H, 64, N]
    out = O.transpose(0, 3, 1, 2).reshape(B, N, NUM_HEADS * DIM_HEAD)
    out = out.reshape(B, GRID, GRID, NUM_HEADS * DIM_HEAD)
    shift = int(np.asarray(inputs['window_size'])) // 2
    out = np.roll(out, shift=(-shift, -shift), axis=(1, 2))
    return out.astype(np.float32)


# revision 17
# speedup vs baseline: 2.0163x; 1.3159x over previous
"""Trainium2 Bass kernel for nn_ABCFramework_17755394802208.

Conv backbone (6x 3x3 SAME convs + 2 maxpools, 256^2 -> 64^2) feeding full
self-attention over N=4096 tokens with a Swin-style relative-position bias.

Sharding: 8 cores = (batch b in {0,1}) x (head h in {0..3}); each core runs the
conv backbone for its batch, projects q/k/v for its head, and computes full
attention for its (b, h). Output slices are gathered/reassembled on host.

Conv formulation: each matmul processes G image rows at once. The moving tile
holds rows (ky, ci, g) x (strip s, padded col xp); the stationary weights are
block-diagonal [3*Cin*G, Cout*G] (one block per g), one stationary per kx tap,
accumulated over kx in PSUM. All matmuls run in bf16.

Attention: logits S^T = K_c^T Q_j in PSUM (bf16 matmul), exp on the Scalar
engine, then the relative-position bias is applied multiplicatively on the
Vector engine: exp(s+b) = exp(s) * EB where EB = exp(bias atlas) is computed
once on-chip. AV accumulates in PSUM with an extra ones-row in V giving the
softmax denominators.
"""
import sys

sys.path.insert(0, '/opt/trn_rl_repo')

import numpy as np

try:
    from ml_dtypes import bfloat16 as BF16_NP
except ImportError:  # pragma: no cover
    import jax.numpy as _jnp
    BF16_NP = _jnp.bfloat16

NUM_HEADS = 4
DIM_HEAD = 64
TABLE_M = 160
B = 2
N = 4096          # tokens (64 x 64)
GRID = 64
NCH = 32          # m-chunks of 128 keys
NQC = 8           # n-chunks of 512 queries
CF = 8192         # 127 * 64 compact-table window length

# conv layer configs: (Cin, Cout, H, W, G rows-per-group)
CONVS = [(1, 3, 256, 256, 42), (3, 3, 256, 256, 14),
         (3, 6, 128, 128, 14), (6, 6, 128, 128, 7),
         (6, 9, 64, 64, 7), (9, 9, 64, 64, 4)]

# plane name -> (C, H, W, slack rows below the bottom guard)
PLANES = {
    'x':  (1, 256, 256, 38),
    'm1': (3, 256, 256, 37),
    'm2': (3, 256, 256, 9),
    'p1': (3, 128, 128, 12),
    'm3': (6, 128, 128, 11),
    'm4': (6, 128, 128, 4),
    'p2': (6, 64, 64, 6),
    'm5': (9, 64, 64, 5),
}
# conv wiring: (layer idx, in plane, out plane or 'tok')
WIRE = [(0, 'x', 'm1'), (1, 'm1', 'm2'), (2, 'p1', 'm3'),
        (3, 'm3', 'm4'), (4, 'p2', 'm5'), (5, 'm5', 'tok')]
POOLS = [('m2', 'p1'), ('m4', 'p2')]
BOTTOM_FIX = {'m1', 'm3', 'm5'}        # planes needing post-layer guard re-zero


def _lay(H, W, slack):
    Wp = W + 2
    Goff = Wp + 1
    rows = H + 2 + slack
    return Wp, Goff, rows * Wp + 2 * Goff  # Wp, guard offset, buflen


def _chunks(Sf, R, W):
    ns_max = 512 // W
    out = []
    s = 0
    while s < Sf:
        ns = min(ns_max, Sf - s)
        out.append((s, ns))
        s += ns
    if R:
        out.append((Sf, 1))
    return out


_BUILD_CACHE = {}


def _build():
    if 'nc' in _BUILD_CACHE:
        return _BUILD_CACHE['nc']
    import concourse.bass as bass
    import concourse.mybir as mybir
    import concourse.tile as tile
    from concourse import bacc

    F32 = mybir.dt.float32
    BF16 = mybir.dt.bfloat16
    AF = mybir.ActivationFunctionType
    ALU = mybir.AluOpType

    nc = bacc.Bacc("TRN2", target_bir_lowering=False, debug=False, num_devices=8)

    # ---- external inputs (per-core shards prepared on host) ----
    _, _, BLX = _lay(*PLANES['x'][1:])
    x_d = nc.dram_tensor("x", [1, BLX], BF16, kind="ExternalInput")
    wk_d, bx_d = [], []
    for i, (ci, co, _, _, G) in enumerate(CONVS):
        rows, cols = 3 * ci * G, co * G
        wk_d.append([nc.dram_tensor(f"w{i}_{kx}", [rows, cols], BF16,
                                    kind="ExternalInput") for kx in range(3)])
        bx_d.append(nc.dram_tensor(f"bx{i}", [cols], F32, kind="ExternalInput"))
    wq_d = nc.dram_tensor("wq", [9, 64], BF16, kind="ExternalInput")
    wkk_d = nc.dram_tensor("wk", [9, 64], BF16, kind="ExternalInput")
    wv_d = nc.dram_tensor("wv", [9, 64], BF16, kind="ExternalInput")
    wa_d = nc.dram_tensor("watlas", [128, CF], BF16, kind="ExternalInput")
    out_d = nc.dram_tensor("out", [64, N], F32, kind="ExternalOutput")

    with tile.TileContext(nc) as tc:
        with tc.tile_pool(name="const", bufs=1) as const, \
             tc.tile_pool(name="work", bufs=2) as work, \
             tc.tile_pool(name="dram", bufs=1, space="DRAM") as dram:

            # ---------------- constants ----------------
            wkt, bxt = [], []
            for i, (ci, co, _, _, G) in enumerate(CONVS):
                rows, cols = 3 * ci * G, co * G
                ceng = nc.sync if i == 0 else nc.gpsimd
                trip = []
                for kx in range(3):
                    t = const.tile([rows, cols], BF16, tag=f"w{i}_{kx}",
                                   name=f"w{i}_{kx}")
                    ceng.dma_start(out=t, in_=wk_d[i][kx][:, :])
                    trip.append(t)
                wkt.append(trip)
                bt = const.tile([cols, 1], F32, tag=f"bx{i}", name=f"bx{i}")
                ceng.dma_start(out=bt, in_=bx_d[i][:, None])
                bxt.append(bt)
            wq_t = const.tile([9, 64], BF16, tag="wq")
            wkk_t = const.tile([9, 64], BF16, tag="wkk")
            wv_t = const.tile([9, 64], BF16, tag="wv")
            nc.gpsimd.dma_start(out=wq_t, in_=wq_d[:, :])
            nc.gpsimd.dma_start(out=wkk_t, in_=wkk_d[:, :])
            nc.gpsimd.dma_start(out=wv_t, in_=wv_d[:, :])

            zeros_bf = const.tile([128, 512], BF16, tag="zbf")
            nc.vector.memset(zeros_bf, 0.0)
            ones_bf = const.tile([1, 64], BF16, tag="obf")
            nc.vector.memset(ones_bf, 1.0)

            # bias atlas -> EB = exp(atlas), computed once on-chip
            Wt = const.tile([128, CF], BF16, tag="W")
            nc.gpsimd.dma_start(out=Wt, in_=wa_d[:, :])
            EB = const.tile([128, CF], BF16, tag="EB")
            nc.scalar.activation(out=EB, in_=Wt, func=AF.Exp)

            # ---------------- DRAM planes + zero scratch ----------------
            geom, plane = {}, {}
            for nm, (C, H, Wd, slack) in PLANES.items():
                geom[nm] = _lay(H, Wd, slack)
                if nm != 'x':
                    plane[nm] = dram.tile([C, geom[nm][2]], BF16, tag=nm, name=nm)
            tok_d = dram.tile([9, N], BF16, tag="tok", name="tok")
            zs = dram.tile([1, 32768], BF16, tag="zs", name="zs")
            nc.sync.dma_start(
                out=bass.AP(tensor=zs.tensor, offset=0, ap=[[256, 128], [1, 256]]),
                in_=zeros_bf[0:128, 0:256])

            def zfill(tensor, offset, ap):
                total = 1
                for _, cnt in ap:
                    total *= cnt
                assert total <= 32768, total
                nc.sync.dma_start(
                    out=bass.AP(tensor=tensor, offset=offset, ap=ap),
                    in_=bass.AP(tensor=zs.tensor, offset=0, ap=[[1, total]]))

            # initial guards: top strip for every on-chip plane; conv writes
            # cover the column guards (padded width), pool-output planes also
            # need columns + bottom (pools never write them)
            for nm in ['m1', 'm2', 'p1', 'm3', 'm4', 'p2', 'm5']:
                C, H, Wd, slack = PLANES[nm]
                Wp, Goff, bl = geom[nm]
                t = plane[nm].tensor
                zfill(t, 0, [[bl, C], [1, Goff + Wp]])
                if nm in ('p1', 'p2'):
                    zfill(t, Goff + Wp, [[bl, C], [Wp, H + 1 + slack]])
                    zfill(t, Goff + 2 * Wp - 1, [[bl, C], [Wp, H + 1 + slack]])
                    zfill(t, Goff + (H + 1) * Wp,
                          [[bl, C], [1, (slack + 1) * Wp + Goff]])

            # ---------------- conv backbone ----------------
            def conv_layer(li, in_nm, out_nm):
                Cin, Cout, H, Wd, G = CONVS[li]
                Wp, Goff, bl = geom[in_nm]
                Sf, R = H // G, H % G
                S_tot = Sf + (1 if R else 0)
                rows, cols = 3 * Cin * G, Cout * G
                in_t = x_d if in_nm == 'x' else plane[in_nm].tensor

                mov = work.tile([rows, S_tot, Wp], BF16, tag="mov", bufs=2,
                                name=f"mov{li}")
                lengs = [nc.sync, nc.scalar, nc.gpsimd]
                for ky in range(3):
                    for ci in range(Cin):
                        src = bass.AP(tensor=in_t,
                                      offset=ci * bl + Goff + ky * Wp,
                                      ap=[[Wp, G], [G * Wp, S_tot], [1, Wp]])
                        pb = (ky * Cin + ci) * G
                        lengs[(ky * Cin + ci) % 3].dma_start(
                            out=mov[pb:pb + G, :, :], in_=src)

                Wdo = Wd + 2  # write padded width so edge guards ride along
                out_t = work.tile([cols, S_tot, Wdo], BF16, tag="out", bufs=2,
                                  name=f"out{li}")
                nc.vector.memset(out_t[:, :, 0:1], 0.0)
                nc.vector.memset(out_t[:, :, Wdo - 1:Wdo], 0.0)
                chs = _chunks(Sf, R, Wd)
                for g0 in range(0, len(chs), 4):
                    grp = chs[g0:g0 + 4]
                    pts = [psc.tile([cols, 512], F32, tag="cps",
                                    name=f"cps{li}_{g0}_{gi}")
                           for gi in range(len(grp))]
                    for kx in range(3):
                        for pt, (s0, ns) in zip(pts, grp):
                            nc.tensor.matmul(pt[:, 0:ns * Wd], wkt[li][kx],
                                             mov[:, s0:s0 + ns, kx:kx + Wd],
                                             start=(kx == 0), stop=(kx == 2))
                    for pt, (s0, ns) in zip(pts, grp):
                        nc.vector.scalar_tensor_tensor(
                            out=out_t[:, s0:s0 + ns, 1:1 + Wd],
                            in0=pt[:, 0:ns * Wd],
                            scalar=bxt[li], in1=zeros_bf[0:cols, 0:ns * Wd],
                            op0=ALU.add, op1=ALU.max)

                wengs = [nc.gpsimd, nc.sync, nc.scalar]
                if out_nm == 'tok':
                    for co in range(Cout):
                        dst = bass.AP(tensor=tok_d.tensor, offset=co * N,
                                      ap=[[64, G], [G * 64, Sf], [1, 64]])
                        wengs[co % 3].dma_start(
                            out=dst,
                            in_=out_t[co * G:(co + 1) * G, 0:Sf, 1:1 + Wd])
                    return
                Wpo, Goffo, blo = geom[out_nm]
                slo = PLANES[out_nm][3]
                ot = plane[out_nm].tensor
                # one write per output channel (padded width covers the column
                # guards); with a partial strip (R>0) the garbage rows g>=R of
                # strip Sf spill into the slack region, re-zeroed below for
                # planes whose consumer reads the guard row
                for co in range(Cout):
                    dst = bass.AP(tensor=ot, offset=co * blo + Goffo + Wpo,
                                  ap=[[Wpo, G], [G * Wpo, S_tot], [1, Wdo]])
                    wengs[co % 3].dma_start(
                        out=dst, in_=out_t[co * G:(co + 1) * G, 0:S_tot, :])
                if out_nm in BOTTOM_FIX:
                    C_, H_, _, _ = PLANES[out_nm]
                    total = (slo + 1) * Wpo + Goffo
                    nc.gpsimd.dma_start(
                        out=bass.AP(tensor=ot, offset=Goffo + (H_ + 1) * Wpo,
                                    ap=[[blo, C_], [1, total]]),
                        in_=bass.AP(tensor=zs.tensor, offset=0,
                                    ap=[[1, C_ * total]]))

            def pool_layer(in_nm, out_nm):
                C, H, Wd, _ = PLANES[in_nm]
                Wp, Goff, bl = geom[in_nm]
                H2, W2 = H // 2, Wd // 2
                Wp2, Goff2, bl2 = geom[out_nm]
                it, ot = plane[in_nm].tensor, plane[out_nm].tensor
                # one load/store per channel: output rows on partitions (H2<=128)
                qs = [nc.sync, nc.scalar, nc.gpsimd]
                for c in range(C):
                    t3 = work.tile([128, 2, Wd], BF16, tag="pool", bufs=3,
                                   name=f"pool_{in_nm}_{c}")
                    src = bass.AP(tensor=it,
                                  offset=c * bl + Goff + Wp + 1,
                                  ap=[[2 * Wp, H2], [Wp, 2], [1, Wd]])
                    qs[c % 3].dma_start(out=t3[0:H2, :, :], in_=src)
                    m1t = work.tile([128, 2, W2], BF16, tag="plw", bufs=2)
                    nc.vector.tensor_max(m1t[0:H2], t3[0:H2, :, 0::2],
                                         t3[0:H2, :, 1::2])
                    m2t = work.tile([128, W2], BF16, tag="plh", bufs=2)
                    nc.vector.tensor_max(m2t[0:H2], m1t[0:H2, 0, :],
                                         m1t[0:H2, 1, :])
                    dst = bass.AP(tensor=ot, offset=c * bl2 + Goff2 + Wp2 + 1,
                                  ap=[[Wp2, H2], [1, W2]])
                    qs[(c + 1) % 3].dma_start(out=dst, in_=m2t[0:H2, :])

            scope_conv = nc.named_scope("conv"); scope_conv.__enter__()
            with tc.tile_pool(name="psc", bufs=8, space="PSUM") as psc:
                conv_layer(0, 'x', 'm1')
                conv_layer(1, 'm1', 'm2')
                pool_layer('m2', 'p1')
                conv_layer(2, 'p1', 'm3')
                conv_layer(3, 'm3', 'm4')
                pool_layer('m4', 'p2')
                conv_layer(4, 'p2', 'm5')
                conv_layer(5, 'm5', 'tok')
            scope_conv.__exit__(None, None, None)

            # ---------------- tokens + q/k/v ----------------
            scope_qkv = nc.named_scope("qkv"); scope_qkv.__enter__()
            tokT = const.tile([9, N], BF16, tag="tok")
            nc.sync.dma_start(out=tokT, in_=tok_d[:, :])

            qT = const.tile([64, N], BF16, tag="qT")
            kT = const.tile([64, N], BF16, tag="kT")
            v_sb = const.tile([128, NCH, 65], BF16, tag="v")
            nc.vector.memset(v_sb, 1.0)

            with tc.tile_pool(name="psq", bufs=2, space="PSUM") as psq:
                for j in range(NQC):
                    ps_q = psq.tile([64, 512], F32, tag="qps")
                    nc.tensor.matmul(ps_q, wq_t, tokT[:, j * 512:(j + 1) * 512],
                                     start=True, stop=True)
                    nc.scalar.activation(out=qT[:, j * 512:(j + 1) * 512],
                                         in_=ps_q, func=AF.Copy,
                                         scale=float(DIM_HEAD) ** -0.5)
                    ps_k = psq.tile([64, 512], F32, tag="kps")
                    nc.tensor.matmul(ps_k, wkk_t, tokT[:, j * 512:(j + 1) * 512],
                                     start=True, stop=True)
                    nc.scalar.activation(out=kT[:, j * 512:(j + 1) * 512],
                                         in_=ps_k, func=AF.Copy)
                for c in range(NCH):
                    ps_v = psq.tile([128, 64], F32, tag="vps")
                    nc.tensor.matmul(ps_v, tokT[:, c * 128:(c + 1) * 128], wv_t,
                                     start=True, stop=True)
                    nc.vector.tensor_copy(v_sb[:, c, 0:64], ps_v)
            scope_qkv.__exit__(None, None, None)

            # ---------------- attention ----------------
            scope_attn = nc.named_scope("attn"); scope_attn.__enter__()
            with tc.tile_pool(name="pss", bufs=2, space="PSUM") as pss, \
                 tc.tile_pool(name="psa", bufs=2, space="PSUM") as psa:
                for j in range(NQC):
                    acc = psa.tile([65, 512], F32, tag="acc")
                    for cg in range(0, NCH, 3):
                        w = min(3, NCH - cg)
                        # S^T for chunks cg..cg+w-1, one 3-bank PSUM tile;
                        # exp over all w*512 columns in a single ACT op
                        s3 = pss.tile([128, 3, 512], F32, tag="s3")
                        for i in range(w):
                            c = cg + i
                            nc.tensor.matmul(s3[:, i, :],
                                             kT[:, c * 128:(c + 1) * 128],
                                             qT[:, j * 512:(j + 1) * 512],
                                             start=True, stop=True)
                        at3 = work.tile([128, 3, 512], BF16, tag="at", bufs=3)
                        nc.scalar.activation(out=at3[:, 0:w, :],
                                             in_=s3[:, 0:w, :], func=AF.Exp)
                        atb3 = work.tile([128, 3, 512], BF16, tag="atb", bufs=3)
                        for i in range(w):
                            c = cg + i
                            s0 = (8 * j - 2 * c + 63) * 64
                            nc.vector.tensor_mul(atb3[:, i, :], at3[:, i, :],
                                                 EB[:, s0:s0 + 512])
                        for i in range(w):
                            c = cg + i
                            nc.tensor.matmul(acc, v_sb[:, c, :], atb3[:, i, :],
                                             start=(c == 0),
                                             stop=(c == NCH - 1))
                    # epilogue: divide by the attention sums (row 64 of acc)
                    sums = work.tile([1, 512], F32, tag="sums", bufs=2)
                    nc.vector.tensor_copy(sums, acc[64:65, :])
                    rcp_f = work.tile([1, 512], F32, tag="rcpf", bufs=2)
                    nc.vector.reciprocal_approx_fast(out=rcp_f, in_=sums)
                    bc_sb = work.tile([64, 512], F32, tag="bcs", bufs=2)
                    nc.gpsimd.partition_broadcast(bc_sb, rcp_f)
                    res = work.tile([64, 512], F32, tag="res", bufs=2)
                    nc.vector.tensor_mul(res, acc[0:64, :], bc_sb)
                    nc.sync.dma_start(out=out_d[:, j * 512:(j + 1) * 512],
                                      in_=res)
            scope_attn.__exit__(None, None, None)

    nc.finalize()
    _BUILD_CACHE['nc'] = nc
    return nc


def _prep_inputs(inputs):
    """Build the 8 per-core input maps (layout/packing only)."""
    x = np.asarray(inputs['x'], dtype=np.float32)
    qkv_w = np.asarray(inputs['qkv_w'], dtype=np.float32)
    table = np.asarray(inputs['bias_table'], dtype=np.float32)

    Wp, Goff, BLX = _lay(*PLANES['x'][1:])
    rows_x = PLANES['x'][1] + 2 + PLANES['x'][3]
    xbufs = []
    for b in range(B):
        pad = np.zeros((rows_x, Wp), np.float32)
        pad[1:257, 1:257] = x[b, 0]
        buf = np.zeros((1, BLX), np.float32)
        buf[0, Goff:Goff + rows_x * Wp] = pad.reshape(-1)
        xbufs.append(buf.astype(BF16_NP))

    wks, bxs = [], []
    for i, (Cin, Cout, _, _, G) in enumerate(CONVS):
        w = np.asarray(inputs[f'conv{i + 1}_w'], dtype=np.float32)
        bias = np.asarray(inputs[f'conv{i + 1}_b'], dtype=np.float32)
        trip = []
        ar = np.arange(G)
        for kx in range(3):
            Wk = np.zeros((3 * Cin * G, Cout * G), np.float32)
            for ky in range(3):
                for ci in range(Cin):
                    r0 = (ky * Cin + ci) * G
                    for co in range(Cout):
                        Wk[r0 + ar, co * G + ar] = w[co, ci, ky, kx]
            trip.append(Wk.astype(BF16_NP))
        wks.append(trip)
        bxs.append(np.repeat(bias, G).astype(np.float32))

    atlases = []
    for h in range(NUM_HEADS):
        tab = table[:, h].reshape(2 * TABLE_M - 1, 2 * TABLE_M - 1)
        Ct = tab[96:96 + 127, 96:96 + 127]  # [127, 127]
        tmp = np.zeros((127, 128), np.float32)
        tmp[:, :127] = Ct
        cfbuf = np.zeros(191 + 16256 + 129, np.float32)
        cfbuf[191:191 + 16256] = tmp.reshape(-1)
        sw = np.lib.stride_tricks.sliding_window_view(cfbuf, 16256)
        p = np.arange(128)
        offs = 254 - (p % 64) - 128 * (p // 64)
        full = sw[offs]                                   # [128, 127*128]
        a2 = full.reshape(128, 127, 128)[:, :, 0:64].reshape(128, 127 * 64)
        atl = np.zeros((128, CF), np.float32)
        atl[:, :127 * 64] = a2
        atlases.append(atl.astype(BF16_NP))

    in_maps = []
    for core in range(8):
        b, h = core // 4, core % 4
        m = {"x": xbufs[b], "watlas": atlases[h]}
        for i in range(6):
            for kx in range(3):
                m[f"w{i}_{kx}"] = wks[i][kx]
            m[f"bx{i}"] = bxs[i]
        m["wq"] = np.ascontiguousarray(
            qkv_w[h * 64:(h + 1) * 64, :].T).astype(BF16_NP)
        m["wk"] = np.ascontiguousarray(
            qkv_w[256 + h * 64:256 + (h + 1) * 64, :].T).astype(BF16_NP)
        m["wv"] = np.ascontiguousarray(
            qkv_w[512 + h * 64:512 + (h + 1) * 64, :].T).astype(BF16_NP)
        in_maps.append(m)
    return in_maps


def kernel(_trace=False, **inputs):
    from concourse.bass_utils import run_bass_kernel_spmd
    nc = _build()
    in_maps = _prep_inputs(inputs)
    import os
    tdir = os.environ.get("KTRACE_DIR")
    if tdir:
        os.makedirs(tdir, exist_ok=True)
    res = run_bass_kernel_spmd(nc, in_maps, core_ids=list(range(8)),
                               trace=_trace, tmpdir=tdir)
    if _trace:
        kernel.last_exec_ns = res.exec_time_ns
        kernel.last_results = res
    # assemble: core -> (b, h): [64(d), 4096(n)]
    O = np.stack([np.stack([res.results[b * 4 + h]["out"] for h in range(4)])
                  for b in range(B)])                      # [B, H, 64, N]
    out = O.transpose(0, 3, 1, 2).reshape(B, N, NUM_HEADS * DIM_HEAD)
    out = out.reshape(B, GRID, GRID, NUM_HEADS * DIM_HEAD)
    shift = int(np.asarray(inputs['window_size'])) // 2
    out = np.roll(out, shift=(-shift, -shift), axis=(1, 2))
    return out.astype(np.float32)
